# revision 16
# baseline (speedup 1.0000x reference)
"""AttentionV1 Trainium2 Bass kernel (v3).

Data-parallel over batch: 8 images -> 8 NeuronCores. Per core:
  qkv = W_qkv @ x            (1x1 conv, PE, bf16, permuted 5-block layout)
  qkv = dwconv3x3(qkv)       (q,k: flat 9-tap STT on DVE at 2x + edge fixups;
                              v: diag-matmul on PE, wrap-free column ranges)
  qf = q*f, kf = k*f         (DVE)
  G  = qf @ kf^T             (PE transpose-via-identity + PE gram)
  attn = softmax(G / (nq nk^T))  (small-tensor phase)
  out = (blockdiag(attn)^T @ W_proj^T)^T @ v   (PE)

Channel blocks (output-channel permutation of W_qkv/taps):
  B0 = q[0:128], B1 = q[128:192] || k[128:192], B2 = k[0:128],
  B3 = v[0:128], B4 = v[128:192]

q,k stencil buffers are FLAT (no column pads) so every DVE op is a
contiguous 1D bf16 stream (2x mode). Horizontal shifts then wrap across
row boundaries; 6 tiny fixup ops per block subtract the wrapped terms
(taps cols 9..17 hold negated taps). sb2 holds the by-one-shifted copy so
dx=+-1 reads stay 4B-aligned.
"""
import sys

for _p in ("/opt/trn_rl_repo",):
    if _p not in sys.path:
        sys.path.insert(0, _p)

import numpy as np

import concourse.bass as bass
import concourse.bacc as bacc
import concourse.mybir as mybir
from concourse.tile import TileContext
from concourse.bass_utils import run_bass_kernel_spmd

F32 = mybir.dt.float32
BF16 = mybir.dt.bfloat16
AL = mybir.AluOpType
AF = mybir.ActivationFunctionType

C = 192          # channels
O = 576          # 3*C
H = 128
W = 128
N = H * W        # 16384
HEADS = 8
CH = 24          # channels per head
TR = 8           # rows per spatial tile
NT = H // TR     # 16 tiles
S = TR * W       # 1024 spatial elems per tile
PR = TR + 2      # rows incl halo
FL = PR * W      # 1280 flat elems
HB = 8           # head/tail pad elems on flat q,k buffers
NCHUNK = S // 128  # 8 transpose chunks per tile

BLK = [128, 128, 128, 128, 64]
QK_BLOCKS = (0, 1, 2)
TAP_OFF = [(3 * (dy + 1) + (dx + 1), dy, dx)
           for dy in (-1, 0, 1) for dx in (-1, 0, 1)]
# DVE tap order: a dx=0 tap first (tensor_scalar overwrite), then the rest
DVE_TAPS = [(1, -1, 0)] + [t for t in TAP_OFF if t[0] != 1]


def build_nc():
    nc = bacc.Bacc()
    x_d = nc.declare_dram_parameter("x", [C, H, W], F32, isOutput=False)
    f_d = nc.declare_dram_parameter("f", [C, H, W], F32, isOutput=False)
    wq_d = nc.declare_dram_parameter("wq", [C, O], BF16, isOutput=False)
    taps_d = nc.declare_dram_parameter("taps", [O, 18], F32, isOutput=False)
    vd3_d = nc.declare_dram_parameter("vdiag3", [128, 9 * 128], BF16, isOutput=False)
    vd4_d = nc.declare_dram_parameter("vdiag4", [64, 9 * 64], BF16, isOutput=False)
    wp_d = nc.declare_dram_parameter("wp", [C, C], BF16, isOutput=False)
    temp_d = nc.declare_dram_parameter("temp", [CH, HEADS], F32, isOutput=False)
    idb_d = nc.declare_dram_parameter("identb", [128, 128], BF16, isOutput=False)
    idf_d = nc.declare_dram_parameter("identf", [128, 128], F32, isOutput=False)
    out_d = nc.declare_dram_parameter("out", [C, N], BF16, isOutput=True)

    with TileContext(nc) as tc:
        with (
            tc.tile_pool(name="const", bufs=1) as cpool,
            tc.tile_pool(name="vstore", bufs=1) as vpool,
            tc.tile_pool(name="xin", bufs=2) as xpool,
            tc.tile_pool(name="fin", bufs=2) as fpool,
            tc.tile_pool(name="qkv", bufs=2) as qkvpool,
            tc.tile_pool(name="st", bufs=2) as stpool,
            tc.tile_pool(name="scr", bufs=2) as scrpool,
            tc.tile_pool(name="tsb", bufs=3) as tsbpool,
            tc.tile_pool(name="fin2", bufs=1) as finpool,
            tc.tile_pool(name="outsb", bufs=3) as outpool,
            tc.tile_pool(name="mm", bufs=2, space="PSUM") as mmpsum,
            tc.tile_pool(name="vps", bufs=1, space="PSUM") as vpsum,
            tc.tile_pool(name="tps", bufs=1, space="PSUM") as tpsum,
            tc.tile_pool(name="gram", bufs=1, space="PSUM") as gpsum,
        ):
            # ---- constants ----
            wq_sb = [cpool.tile([128, O], BF16, tag="wq0", name="wq0"),
                     cpool.tile([64, O], BF16, tag="wq1", name="wq1")]
            nc.sync.dma_start(out=wq_sb[0][:], in_=wq_d[0:128, :])
            nc.sync.dma_start(out=wq_sb[1][:], in_=wq_d[128:192, :])
            taps_sb = []
            ms = 0
            for bi, psz in enumerate(BLK):
                tt = cpool.tile([psz, 18], F32, tag=f"taps{bi}", name=f"taps{bi}")
                nc.sync.dma_start(out=tt[:], in_=taps_d[ms:ms + psz, :])
                taps_sb.append(tt)
                ms += psz
            vd3 = cpool.tile([128, 9 * 128], BF16, tag="vd3", name="vd3")
            nc.sync.dma_start(out=vd3[:], in_=vd3_d[:])
            vd4 = cpool.tile([64, 9 * 64], BF16, tag="vd4", name="vd4")
            nc.sync.dma_start(out=vd4[:], in_=vd4_d[:])
            wp_sb = [cpool.tile([96, C], BF16, tag="wp0", name="wp0"),
                     cpool.tile([96, C], BF16, tag="wp1", name="wp1")]
            nc.sync.dma_start(out=wp_sb[0][:], in_=wp_d[0:96, :])
            nc.sync.dma_start(out=wp_sb[1][:], in_=wp_d[96:192, :])
            temp_sb = cpool.tile([CH, HEADS], F32, tag="temp", name="temp")
            nc.sync.dma_start(out=temp_sb[:], in_=temp_d[:])
            identb = cpool.tile([128, 128], BF16, tag="identb", name="identb")
            nc.sync.dma_start(out=identb[:], in_=idb_d[:])
            identf = cpool.tile([128, 128], F32, tag="identf", name="identf")
            nc.sync.dma_start(out=identf[:], in_=idf_d[:])

            v_sb = [vpool.tile([128, N], BF16, tag="v0", name="v0"),
                    vpool.tile([64, N], BF16, tag="v1", name="v1")]
            sq_sb = [cpool.tile([128, NT], F32, tag=f"sq{i}", name=f"sq{i}")
                     for i in range(3)]
            # gram accumulators packed into one PSUM bank
            g_all = gpsum.tile([128, 512], F32, tag="g", name="g")
            g_ps = [g_all[:, 0:C], g_all[0:64, 256:256 + C]]

            for t in range(NT):
                r0 = t * TR
                # ---- load x tile with halo rows (bf16 cast via gpsimd dma) ----
                xt = [xpool.tile([128, FL], BF16, tag="x0", name="x0"),
                      xpool.tile([64, FL], BF16, tag="x1", name="x1")]
                lo = r0 - 1
                hi = r0 + TR + 1
                dlo = max(lo, 0)
                dhi = min(hi, H)
                off = dlo - lo
                for ci, (cs, cp) in enumerate(((0, 128), (128, 64))):
                    if lo < 0:
                        nc.vector.memset(xt[ci][:, 0:W], 0.0)
                    if hi > H:
                        nc.vector.memset(xt[ci][:, (PR - 1) * W:FL], 0.0)
                    nc.gpsimd.dma_start(
                        out=xt[ci][:, off * W:(off + dhi - dlo) * W],
                        in_=x_d[cs:cs + cp, dlo:dhi, :],
                    )
                ft_a = fpool.tile([128, S], BF16, tag="fa", name="fa")
                nc.gpsimd.dma_start(out=ft_a[:], in_=f_d[0:128, r0:r0 + TR, :])
                ft_b = fpool.tile([128, S], BF16, tag="fb", name="fb")
                nc.gpsimd.dma_start(out=ft_b[0:64, :], in_=f_d[128:192, r0:r0 + TR, :])
                nc.gpsimd.dma_start(out=ft_b[64:128, :], in_=f_d[128:192, r0:r0 + TR, :])

                # ---- qkv matmul (5 blocks, 10 halo rows) + flat psum->sbuf ----
                sb = []    # flat buffers: q,k [psz, HB+FL+HB]; v [psz, FL]
                sb2 = []
                ms = 0
                for bi, psz in enumerate(BLK):
                    qk = bi in QK_BLOCKS
                    if qk:
                        b1 = qkvpool.tile([psz, 2 * HB + FL], BF16,
                                          tag=f"sb{bi}", name=f"sb{bi}")
                        b2 = qkvpool.tile([psz, 2 * HB + FL], BF16,
                                          tag=f"sc{bi}", name=f"sc{bi}")
                        # zero the sb2 pads the shifted taps read
                        nc.vector.memset(b2[:, 0:HB], 0.0)
                        nc.vector.memset(b2[:, HB + FL - 1:], 0.0)
                        sb.append(b1)
                        sb2.append(b2)
                    else:
                        b1 = qkvpool.tile([psz, FL], BF16, tag=f"sb{bi}",
                                          name=f"sb{bi}")
                        sb.append(b1)
                        sb2.append(None)
                    base = HB if qk else 0
                    # one 2.5-bank psum tile for all 10 rows -> single copies
                    ps = mmpsum.tile([psz, FL], F32, tag="mmpa", name="mmpa",
                                     bufs=1)
                    for c0, csz in ((0, 4), (4, 4), (8, 2)):
                        L = csz * W
                        nc.tensor.matmul(
                            ps[:, c0 * W:c0 * W + L],
                            wq_sb[0][:, ms:ms + psz],
                            xt[0][:, c0 * W:c0 * W + L],
                            start=True, stop=False)
                        nc.tensor.matmul(
                            ps[:, c0 * W:c0 * W + L],
                            wq_sb[1][:, ms:ms + psz],
                            xt[1][:, c0 * W:c0 * W + L],
                            start=False, stop=True)
                    nc.scalar.activation(
                        b1[:, base:base + FL], ps[:], AF.Copy)
                    if qk:
                        # sb2[i] = sb[i+1]
                        nc.scalar.activation(
                            b2[:, HB - 1:HB - 1 + FL], ps[:], AF.Copy)
                    ms += psz

                # ---- q,k stencil on DVE (flat 2x ops + wrap fixups) ----
                st = []
                for bi in QK_BLOCKS:
                    psz = BLK[bi]
                    acc = stpool.tile([psz, S], BF16, tag=f"st{bi}", name=f"st{bi}")
                    prod = stpool.tile([psz, S], BF16, tag=f"pr{bi}",
                                       name=f"pr{bi}")
                    # STT has no 2x uop; tensor_scalar (4x) + tensor_tensor
                    # (2x) pairs are ~30% faster per tap.
                    for idx, (ti, dy, dx) in enumerate(DVE_TAPS):
                        if dx == 0:
                            src = sb[bi][:, HB + (1 + dy) * W:
                                         HB + (1 + dy) * W + S]
                        elif dx == 1:
                            src = sb2[bi][:, HB + (1 + dy) * W:
                                          HB + (1 + dy) * W + S]
                        else:
                            src = sb2[bi][:, HB + (1 + dy) * W - 2:
                                          HB + (1 + dy) * W - 2 + S]
                        w_ap = taps_sb[bi][:, ti:ti + 1]
                        if idx == 0:
                            nc.vector.tensor_scalar_mul(acc[:], src, w_ap)
                        else:
                            nc.vector.tensor_scalar_mul(prod[:], src, w_ap)
                            nc.vector.tensor_add(acc[:], acc[:], prod[:])
                    # wrap fixups: subtract the wrapped horizontal terms.
                    # f3 = data rows view [psz, 10, 128]
                    a3 = acc.rearrange("p (r w) -> p r w", w=W)
                    f3 = sb[bi][:, HB:HB + FL].rearrange(
                        "p (r w) -> p r w", w=W)
                    for dy in (-1, 0, 1):
                        # dx=-1 wrapped at x=0: val = f3[dy+r, 127]
                        # (r=0 with dy=-1 hit the zero head pad: skip)
                        ti = 3 * (dy + 1)
                        ra = 1 if dy == -1 else 0
                        nc.vector.scalar_tensor_tensor(
                            a3[:, ra:TR, 0:1],
                            f3[:, ra + dy:TR + dy, W - 1:W],
                            taps_sb[bi][:, 9 + ti:10 + ti],
                            a3[:, ra:TR, 0:1], op0=AL.mult, op1=AL.add)
                        # dx=+1 wrapped at x=127: val = f3[2+dy+r, 0]
                        # (r=7 with dy=+1 hit the zero tail pad: skip)
                        ti = 3 * (dy + 1) + 2
                        rz = TR - 1 if dy == 1 else TR
                        nc.vector.scalar_tensor_tensor(
                            a3[:, 0:rz, W - 1:W],
                            f3[:, 2 + dy:2 + dy + rz, 0:1],
                            taps_sb[bi][:, 9 + ti:10 + ti],
                            a3[:, 0:rz, W - 1:W], op0=AL.mult, op1=AL.add)
                    st.append(acc)

                # ---- qf/kf multiply (in place) + squares ----
                fts = [ft_a, ft_b, ft_a]
                for i, bi in enumerate(QK_BLOCKS):
                    nc.vector.tensor_mul(st[i][:], st[i][:], fts[i][:])
                    scr = scrpool.tile([128, S], BF16, tag=f"scr{i}",
                                       name=f"scr{i}")
                    nc.scalar.activation(
                        scr[:], st[i][:], AF.Square,
                        accum_out=sq_sb[i][:, t:t + 1])

                # ---- transposes + gram, v-stencil interleaved as PE filler ----
                vps = {}
                for j in range(NCHUNK):
                    g = t * NCHUNK + j
                    col = slice(j * 128, (j + 1) * 128)
                    # qt cols 0:192 = [q_lo | q_hi], kt cols 192:384 = [k_lo | k_hi]
                    qkt = tpsum.tile([128, 384], F32, tag="qkt", name="qkt")
                    nc.tensor.matmul(qkt[:, 0:128], st[0][:, col],
                                     identb[:], start=True, stop=True)
                    nc.tensor.matmul(qkt[:, 128:192], st[1][0:64, col],
                                     identb[0:64, 0:64], start=True, stop=True)
                    nc.tensor.matmul(qkt[:, 192:320], st[2][:, col],
                                     identb[:], start=True, stop=True)
                    nc.tensor.matmul(qkt[:, 320:384], st[1][64:128, col],
                                     identb[64:128, 64:128], start=True, stop=True)
                    qkt_sb = tsbpool.tile([128, 384], BF16, tag="qkts",
                                          name="qkts")
                    nc.scalar.activation(qkt_sb[:], qkt[:], AF.Copy)
                    # --- v-stencil filler: wrap-free column-split matmuls ---
                    half = j // 4
                    jj = j % 4
                    if jj == 0:
                        vps["vp3"] = vpsum.tile([128, 512], F32, tag="vp3",
                                                name="vp3")
                        vps["vp4"] = vpsum.tile([64, 512], F32, tag="vp4",
                                                name="vp4")
                    tap_sl = ((0, 2), (2, 4), (4, 6), (6, 9))[jj]
                    for ti in range(tap_sl[0], tap_sl[1]):
                        _, dy, dx = TAP_OFF[ti]
                        rr = 1 + dy + 4 * half
                        for vbi, vp, vd, vsz in ((3, vps["vp3"], vd3, 128),
                                                 (4, vps["vp4"], vd4, 64)):
                            lhs = vd[:, ti * vsz:(ti + 1) * vsz]
                            v3 = sb[vbi].rearrange("p (r w) -> p r w", w=W)
                            vpd = vp.rearrange("p (r w) -> p r w", w=W)
                            if dx == 0:
                                rhs = v3[:, rr:rr + 4, :]
                                dst = vpd[:, 0:4, :]
                            elif dx == 1:
                                rhs = v3[:, rr:rr + 4, 1:W]
                                dst = vpd[:, 0:4, 0:W - 1]
                            else:
                                rhs = v3[:, rr:rr + 4, 0:W - 1]
                                dst = vpd[:, 0:4, 1:W]
                            nc.tensor.matmul(dst, lhs, rhs,
                                             start=(ti == 0), stop=(ti == 8))
                    if jj == 3:
                        cdst = slice(t * S + half * 512,
                                     t * S + (half + 1) * 512)
                        nc.scalar.activation(v_sb[0][:, cdst], vps["vp3"][:],
                                             AF.Copy)
                        nc.scalar.activation(v_sb[1][:, cdst], vps["vp4"][:],
                                             AF.Copy)
                    # --- gram accumulation ---
                    nc.tensor.matmul(
                        g_ps[0], qkt_sb[:, 0:128], qkt_sb[:, C:2 * C],
                        start=(g == 0), stop=(g == NT * NCHUNK - 1))
                    nc.tensor.matmul(
                        g_ps[1], qkt_sb[:, 128:192], qkt_sb[:, C:2 * C],
                        start=(g == 0), stop=(g == NT * NCHUNK - 1))

            # ================= final small-tensor phase =================
            rb = []
            for i in range(3):
                sq1 = finpool.tile([128, 1], F32, tag=f"sq1_{i}", name=f"sq1_{i}")
                nc.vector.tensor_reduce(
                    sq1[:], sq_sb[i][:], axis=mybir.AxisListType.X, op=AL.add)
                nc.vector.tensor_scalar_max(sq1[:], sq1[:], 1e-24)
                nq = finpool.tile([128, 1], F32, tag=f"nq_{i}", name=f"nq_{i}")
                nc.scalar.activation(nq[:], sq1[:], AF.Sqrt)
                r = finpool.tile([128, 1], F32, tag=f"rq_{i}", name=f"rq_{i}")
                nc.vector.reciprocal(r[:], nq[:])
                rb.append(r)
            # rq rows: q0:128 = rb[0], q128:192 = rb[1][0:64]
            # rk rows: k0:128 = rb[2], k128:192 = rb[1][64:128]

            G_sb = [finpool.tile([128, C], F32, tag="G0", name="G0"),
                    finpool.tile([64, C], F32, tag="G1", name="G1")]
            nc.vector.tensor_scalar_mul(G_sb[0][:], g_ps[0][:], rb[0][:])
            nc.vector.tensor_scalar_mul(G_sb[1][:], g_ps[1][:], rb[1][0:64, :])

            gt0_t = vpsum.tile([128, 512], F32, tag="vp3", name="gt0")
            gt1_t = vpsum.tile([64, 512], F32, tag="vp4", name="gt1")
            gt_ps = [gt0_t[:, 0:C], gt1_t[:, 0:C]]
            nc.tensor.matmul(gt_ps[0][:, 0:128], G_sb[0][:, 0:128], identf[:],
                             is_transpose=True, start=True, stop=True)
            nc.tensor.matmul(gt_ps[0][:, 128:192], G_sb[1][:, 0:128],
                             identf[0:64, 0:64], is_transpose=True,
                             start=True, stop=True)
            nc.tensor.matmul(gt_ps[1][:, 0:128], G_sb[0][:, 128:192], identf[:],
                             is_transpose=True, start=True, stop=True)
            nc.tensor.matmul(gt_ps[1][:, 128:192], G_sb[1][:, 128:192],
                             identf[0:64, 0:64], is_transpose=True,
                             start=True, stop=True)

            rkp = finpool.tile([32, HEADS], F32, tag="rkp", name="rkp")
            nc.vector.memset(rkp[:], 0.0)
            for h in range(HEADS):
                a0 = h * CH
                a1 = a0 + CH
                if a1 <= 128:
                    nc.sync.dma_start(out=rkp[0:CH, h:h + 1],
                                      in_=rb[2][a0:a1, :])
                elif a0 >= 128:
                    nc.sync.dma_start(out=rkp[0:CH, h:h + 1],
                                      in_=rb[1][64 + a0 - 128:64 + a1 - 128, :])
                else:
                    m = 128 - a0
                    nc.sync.dma_start(out=rkp[0:m, h:h + 1],
                                      in_=rb[2][a0:128, :])
                    nc.sync.dma_start(out=rkp[m:CH, h:h + 1],
                                      in_=rb[1][64:64 + a1 - 128, :])
            nc.vector.tensor_mul(rkp[0:CH, :], rkp[0:CH, :], temp_sb[:])

            gt_sb = [finpool.tile([128, C], F32, tag="gts0", name="gts0"),
                     finpool.tile([64, C], F32, tag="gts1", name="gts1")]
            nc.vector.tensor_copy(gt_sb[0][:], gt_ps[0][:])
            nc.vector.tensor_copy(gt_sb[1][:], gt_ps[1][:])
            at = finpool.tile([32, HEADS * 32], F32, tag="at", name="at")
            nc.vector.memset(at[:], 0.0)
            for h in range(HEADS):
                a0 = h * CH
                a1 = a0 + CH
                col = slice(a0, a1)
                if a1 <= 128:
                    nc.sync.dma_start(out=at[0:CH, h * 32:h * 32 + CH],
                                      in_=gt_sb[0][a0:a1, col])
                elif a0 >= 128:
                    nc.sync.dma_start(out=at[0:CH, h * 32:h * 32 + CH],
                                      in_=gt_sb[1][a0 - 128:a1 - 128, col])
                else:
                    m = 128 - a0
                    nc.sync.dma_start(out=at[0:m, h * 32:h * 32 + CH],
                                      in_=gt_sb[0][a0:128, col])
                    nc.sync.dma_start(out=at[m:CH, h * 32:h * 32 + CH],
                                      in_=gt_sb[1][0:a1 - 128, col])
                nc.vector.tensor_scalar_mul(
                    at[0:CH, h * 32:h * 32 + CH],
                    at[0:CH, h * 32:h * 32 + CH],
                    rkp[0:CH, h:h + 1])

            a_sb = finpool.tile([32, HEADS * 32], F32, tag="a", name="a")
            nc.vector.transpose(a_sb[:], at[:])
            e_sb = finpool.tile([32, HEADS * 32], F32, tag="e", name="e")
            nc.scalar.activation(e_sb[:], a_sb[:], AF.Exp)
            e3 = e_sb.rearrange("p (h d) -> p h d", d=32)
            sums = finpool.tile([CH, HEADS], F32, tag="sums", name="sums")
            nc.vector.tensor_reduce(
                sums[:], e3[0:CH, :, 0:CH], axis=mybir.AxisListType.X, op=AL.add)
            rs = finpool.tile([CH, HEADS], F32, tag="rs", name="rs")
            nc.vector.reciprocal(rs[:], sums[:])
            attn = finpool.tile([CH, HEADS * CH], BF16, tag="attn", name="attn")
            for h in range(HEADS):
                nc.vector.tensor_scalar_mul(
                    attn[:, h * CH:(h + 1) * CH],
                    e_sb[0:CH, h * 32:h * 32 + CH],
                    rs[:, h:h + 1])

            bd = [finpool.tile([96, C], BF16, tag="bd0", name="bd0"),
                  finpool.tile([96, C], BF16, tag="bd1", name="bd1")]
            nc.vector.memset(bd[0][:], 0.0)
            nc.vector.memset(bd[1][:], 0.0)
            for h in range(HEADS):
                nc.sync.dma_start(
                    out=bd[h // 4][(h % 4) * CH:(h % 4) * CH + CH,
                                   h * CH:(h + 1) * CH],
                    in_=attn[:, h * CH:(h + 1) * CH])
            # MT: rows = v channels; mt0 and mt1 packed in one psum bank
            mt_t = tpsum.tile([128, 384], F32, tag="qkt", name="mt")
            mt_ps = [mt_t[:, 0:C], mt_t[0:64, C:2 * C]]
            for mi, msl in enumerate((slice(0, 128), slice(128, 192))):
                for k in range(2):
                    nc.tensor.matmul(mt_ps[mi], bd[k][:, msl], wp_sb[k][:],
                                     start=(k == 0), stop=(k == 1))
            mt_sb = [finpool.tile([128, C], BF16, tag="mt_sb0", name="mt_sb0"),
                     finpool.tile([64, C], BF16, tag="mt_sb1", name="mt_sb1")]
            nc.vector.tensor_copy(mt_sb[0][:], mt_ps[0])
            nc.vector.tensor_copy(mt_sb[1][:], mt_ps[1])

            # ---- output: out = MT^T @ v ----
            for j in range(N // 512):
                col = slice(j * 512, (j + 1) * 512)
                for mi, (msz, msl) in enumerate(((128, slice(0, 128)),
                                                 (64, slice(128, 192)))):
                    ps = mmpsum.tile([msz, 512], F32, tag="mmps", name="mmps",
                                     bufs=1)
                    nc.tensor.matmul(ps[:], mt_sb[0][:, msl], v_sb[0][:, col],
                                     start=True, stop=False)
                    nc.tensor.matmul(ps[:], mt_sb[1][:, msl], v_sb[1][:, col],
                                     start=False, stop=True)
                    osb = outpool.tile([msz, 512], BF16, tag=f"osb{mi}",
                                       name=f"osb{mi}")
                    nc.scalar.activation(osb[:], ps[:], AF.Copy)
                    cs = 0 if mi == 0 else 128
                    nc.sync.dma_start(out=out_d[cs:cs + msz, col], in_=osb[:])
    nc.finalize()
    return nc


_NC_CACHE = {}


def _perm():
    return (list(range(0, 128)) + list(range(128, 192))
            + list(range(320, 384)) + list(range(192, 320))
            + list(range(384, 576)))


def kernel(x, feature, W_qkv, W_dw, W_proj, temperature):
    import ml_dtypes
    b = x.shape[0]
    perm = _perm()
    wq_p = np.asarray(W_qkv, np.float32)[perm, :]
    wq = np.ascontiguousarray(wq_p.T).astype(ml_dtypes.bfloat16)
    taps9 = np.asarray(W_dw, np.float32).reshape(O, 9)[perm, :]
    taps = np.ascontiguousarray(np.concatenate([taps9, -taps9], axis=1))
    vtaps = taps9[384:576, :]
    vd3 = np.zeros((128, 9 * 128), np.float32)
    for ti in range(9):
        vd3[:, ti * 128:(ti + 1) * 128][np.arange(128), np.arange(128)] = \
            vtaps[0:128, ti]
    vd4 = np.zeros((64, 9 * 64), np.float32)
    for ti in range(9):
        vd4[:, ti * 64:(ti + 1) * 64][np.arange(64), np.arange(64)] = \
            vtaps[128:192, ti]
    wp = np.ascontiguousarray(np.asarray(W_proj, np.float32).T).astype(
        ml_dtypes.bfloat16)
    temp = np.broadcast_to(
        np.asarray(temperature, np.float32).reshape(1, HEADS), (CH, HEADS))
    temp = np.ascontiguousarray(temp)

    if "nc" not in _NC_CACHE:
        _NC_CACHE["nc"] = build_nc()
    nc = _NC_CACHE["nc"]

    in_maps = []
    for i in range(b):
        in_maps.append({
            "x": np.ascontiguousarray(np.asarray(x[i], np.float32)),
            "f": np.ascontiguousarray(np.asarray(feature[i], np.float32)),
            "wq": wq, "taps": taps,
            "vdiag3": vd3.astype(ml_dtypes.bfloat16),
            "vdiag4": vd4.astype(ml_dtypes.bfloat16),
            "wp": wp, "temp": temp,
            "identb": np.eye(128, dtype=np.float32).astype(ml_dtypes.bfloat16),
            "identf": np.eye(128, dtype=np.float32),
        })
    res = run_bass_kernel_spmd(nc, in_maps, list(range(b)))
    outs = [np.asarray(r["out"], np.float32).reshape(C, H, W)
            for r in res.results]
    return np.stack(outs, axis=0)
